# revision 2
# baseline (speedup 1.0000x reference)
"""AttnBlock (GroupNorm + single-head spatial attention + residual) on 8
Trainium2 NeuronCores.

Sharding: data-parallel over B (4 batches) x 2-way query-sequence parallel =
8 shards. Each core gets the full x[b] (rolled so its query half is the
first 2048 spatial positions), computes GroupNorm + attention for its 2048
queries + residual, and writes a [512, 2048] slice of the output.

Algebraic restructure (vs the q/k/v/out-proj formulation): softmax is
invariant to per-query score offsets and normalization commutes with Wo, so
    scores[q,s] = h_q^T (Wq^T Wk) h_s   (+ (Wk^T bq)^T h_s, zero here)
    out[:,q]    = (sum_s e[s,q] * (Wo Wv) h_s) / Z[q] + (Wo bv + bo) + x[:,q]
Precomputing M = Wq^T Wk and Wov = Wo Wv host-side (512x512 each) removes
the entire Q projection and the entire output projection from the device:
only two h-projections remain (k' = M h, v' = Wov h), and the attention
accumulator po in PSUM f32 is normalized and written out directly.

Compute layout (per core, C=512, S=4096, Sq=2048):
  x        [c, s]   4 chunks of [128, 4096] f16 (channels on partitions;
                    doubles as the residual — f16 rounding of x adds
                    ~5e-4 relative error, far under the 2e-2 gate)
  h = GN(x)         fp8, block-interleaved [p, u, s-block, j, col] so every
                    matmul can run fp8 DoubleRow (pair dim j at 512B stride)
  k' = M h          same interleaved fp8 layout, full S
  vT' = h^T Wov^T   32 tiles of [128, 512] fp8 (spatial on partitions)
  scoresT[s,q] = k'^T h_q  per (128-key-tile x 512-query-block) in PSUM —
                 fp8 DoubleRow, 2 instructions per tile (was 4 fp16).
                 exp()'d on ScalarE into fp8 (x 2^-4 so it can't overflow).
                 Key loop software-pipelined (scores/exp of tile t+1 before
                 the AV matmuls of tile t) so the PE never waits on exp.
  po[c,q] += vT'^T e   accumulated over all 32 key tiles in 4 PSUM banks
  Z[q]    += ones^T e  (full 128-wide fp8 ones lhsT: fast weight load AND
                 broadcasts Z across partitions for free)
  out = x[:, :2048] + po * (1/Z) + bo'   -- the 2^-4 exp shift cancels
                 between po and Z; the reciprocal+muls run on DVE off the
                 PE critical path; output written f16 (host casts to f32).

All matmuls are fp8e4m3 with perf_mode=DoubleRow: two 128-rows of
contraction per PE pass, so each [256-contraction x 512-col] instruction
costs ~216ns — the PE runs at its fp8 peak throughout. fp8 noise on the
scores side averages across the 512-wide contraction and the softmax; on
the value side across 4096 keys. numpy-mirror-predicted end-to-end error
~3e-3 of absmax (measured 6.5e-3 for the previous fp16-scores variant).

GroupNorm: per-quarter [sum, sumsq] pipelined with the x DMAs (sumsq on DVE
via scalar_tensor_tensor+accum, sum on ScalarE via Identity+accum), group
reduce/broadcast across the 16 channels of a group via tiny indicator
matmuls (pre-scaled host-side), rstd via exp(-0.5*ln(var+eps)) (stays in
the one preloaded ACT table set) plus a Newton step.

DMA discipline: the engines service all enqueued transfers round-robin
concurrently, so x chunks are stacked FIFO across the HW queues
(chunk-major) to finish in order; weights queue behind x; small constants
ride the SW queues.
"""
import numpy as np

import bass_rust
import concourse.bass as bass
import concourse.tile as tile
from concourse import mybir
from concourse.bass_utils import run_bass_kernel_spmd

F32 = mybir.dt.float32
F16 = mybir.dt.float16
F8 = mybir.dt.float8e4
AF = mybir.ActivationFunctionType
ALU = mybir.AluOpType

B, C, H, W = 4, 512, 64, 64
S = H * W            # 4096 spatial positions (keys)
SQ = S // 2          # 2048 queries per core
CC = C // 128        # 4 channel chunks
ST = S // 128        # 32 key tiles
SB = S // 512        # 8 column blocks
QB = SQ // 512       # 4 query blocks
NG = 32              # groups
GS = C // NG         # 16 channels per group
EPS = 1e-6
SCALE = 1.0 / float(np.sqrt(C))
E8SHIFT = -4.0 * float(np.log(2.0))  # exp() pre-shift: e*2^-4 fits fp8e4m3
DR = mybir.MatmulPerfMode.DoubleRow


def _split_excess_waits(nc, max_waits=1):
    """walrus in this toolchain rejects instructions with >1 sync-wait.
    Hoist excess waits onto same-engine NOPs placed just before the
    instruction (engine streams are in-order, so this is equivalent)."""
    for f in nc.m.functions:
        for bb in f.blocks:
            out = []
            for inst in bb.instructions:
                si = inst.sync_info
                if si is not None and len(si.on_wait) > max_waits:
                    waits = list(si.on_wait)
                    plain = [w for w in waits if w.wait_reg is None]
                    special = [w for w in waits if w.wait_reg is not None]
                    n_keep = max(0, max_waits - len(special))
                    hoist = plain[: len(plain) - n_keep] if n_keep < len(plain) else []
                    keep = plain[len(hoist):] + special
                    if len(keep) > max_waits:
                        out.append(inst)
                        continue
                    for j, w in enumerate(hoist):
                        nop = mybir.InstNoOp(name=f"{inst.name}-wsplit{j}")
                        nop.engine = inst.engine
                        nop.sync_info = bass_rust.SyncInfo(on_wait=[w], on_update=[])
                        out.append(nop)
                    inst.sync_info = bass_rust.SyncInfo(
                        on_wait=keep, on_update=list(si.on_update))
                out.append(inst)
            bb.instructions = out


def _build():
    nc = bass.Bass(trn_type="TRN2")

    x_d = nc.dram_tensor("x16", [C, S], F16, kind="ExternalInput")
    w8_d = {n: nc.dram_tensor(n, [128, 2, 2, C], F8, kind="ExternalInput")
            for n in ("w8m", "w8ov")}
    bo_d = nc.dram_tensor("boc", [128, CC], F32, kind="ExternalInput")
    ga_d = nc.dram_tensor("gammac", [128, CC], F32, kind="ExternalInput")
    be_d = nc.dram_tensor("betac", [128, CC], F32, kind="ExternalInput")
    ind_d = nc.dram_tensor("ind", [128, CC, NG], F32, kind="ExternalInput")
    indT_d = nc.dram_tensor("indT", [NG, CC, 128], F32, kind="ExternalInput")
    out_d = nc.dram_tensor("out", [CC, 128, SQ], F16, kind="ExternalOutput")

    with tile.TileContext(nc) as tc:
        from contextlib import ExitStack
        with ExitStack() as stack:
            const = stack.enter_context(tc.tile_pool(name="const", bufs=1))
            work = stack.enter_context(tc.tile_pool(name="work", bufs=3))
            p_x = stack.enter_context(tc.tile_pool(name="p_x", bufs=1))
            p_h = stack.enter_context(tc.tile_pool(name="p_h", bufs=1))

            w8_sb = {}
            for n in ("w8m", "w8ov"):
                w8_sb[n] = const.tile([128, 2, 2, C], F8, name=f"{n}_sb")

            def emit_weight_dmas():
                # behind x on the HW queues: x keeps full HBM bandwidth and
                # the weights still land well before the projections run
                for n in ("w8m", "w8ov"):
                    nc.sync.dma_start(out=w8_sb[n][:], in_=w8_d[n][:, :, :, :])

            bo_sb = const.tile([128, CC], F32, name="bo_sb")
            nc.gpsimd.dma_start(out=bo_sb[:], in_=bo_d[:, :])
            ga_sb = const.tile([128, CC], F32, name="ga_sb")
            nc.gpsimd.dma_start(out=ga_sb[:], in_=ga_d[:, :])
            be_sb = const.tile([128, CC], F32, name="be_sb")
            nc.gpsimd.dma_start(out=be_sb[:], in_=be_d[:, :])
            ind_sb = const.tile([128, CC, NG], F32, name="ind_sb")
            nc.gpsimd.dma_start(out=ind_sb[:], in_=ind_d[:, :, :])
            indT_sb = const.tile([NG, CC, 128], F32, name="indT_sb")
            nc.gpsimd.dma_start(out=indT_sb[:], in_=indT_d[:, :, :])

            # full-width ones pair-tile for the DoubleRow Z matmul: its
            # PSUM output is Z broadcast across all 128 partitions for free
            ones8 = const.tile([128, 2, 128], F8, name="ones8")
            nc.vector.memset(ones8[:], 1.0)
            e8b_sb = const.tile([128, 1], F32, name="e8b_sb")
            nc.vector.memset(e8b_sb[:], E8SHIFT)
            eps_sb = const.tile([NG, 1], F32, name="eps_sb")
            nc.vector.memset(eps_sb[:], EPS)

            h8 = p_h.tile([128, 2, SB, 2, 512], F8, name="h8")
            kp8 = p_h.tile([128, 2, SB, 2, 512], F8, name="kp8")
            vT8 = p_h.tile([128, ST, C], F8, name="vT8")
            xc = p_x.tile([128, CC, S], F16, name="xc")

            # warm the ScalarE natural_log_exp table set while the input DMAs
            # are still in flight (the set load is ~2.7us and all ACT
            # functions used below -- Ln/Exp/Identity/Copy -- live in it)
            warm = work.tile([1, 2], F32, name="warm", tag="warm")
            nc.vector.memset(warm[:], 0.0)
            nc.scalar.activation(warm[:, 1:2], warm[:, 0:1], AF.Exp)

            # =========== Phase 1: load x + GroupNorm ===========
            with tc.tile_pool(name="p_gn", bufs=1) as p_gn, \
                 tc.tile_pool(name="ps_gn", bufs=2, space="PSUM") as ps_gn:
                # x in f16: GN stats and h are f16-precision anyway, and
                # halving the critical-path bytes halves time-to-compute.
                # Chunks stacked FIFO across the HW queues (chunk-major) so
                # they finish in order, early.
                for i in range(CC):
                    for qq in range(4):
                        cols = slice(qq * 1024, (qq + 1) * 1024)
                        nc.sync.dma_start(out=xc[:, i, cols],
                                          in_=x_d[i * 128:(i + 1) * 128, cols])
                emit_weight_dmas()

                # per-channel [sum, sumsq], computed PER QUARTER so the stats
                # pipeline with the x DMAs instead of waiting for full
                # chunks. sumsq on DVE ((x*1)*x via scalar_tensor_tensor +
                # accum_out), sum on ScalarE (Identity + accum_out) -- the
                # two run in parallel and neither needs a new ACT table set.
                stats2 = []
                for i in range(CC):
                    s2q = work.tile([128, 2, 4], F32, name="s2q",
                                    tag="gn_s2q", bufs=4)
                    for qq in range(4):
                        qcols = slice(qq * 1024, (qq + 1) * 1024)
                        sq = p_gn.tile([128, 1024], F16, name="sq", tag="sq",
                                       bufs=2)
                        nc.vector.scalar_tensor_tensor(
                            out=sq[:], in0=xc[:, i, qcols], scalar=1.0,
                            in1=xc[:, i, qcols], op0=ALU.mult, op1=ALU.mult,
                            accum_out=s2q[:, 1, qq:qq + 1])
                        sq2 = p_gn.tile([128, 1024], F16, name="sq2",
                                        tag="sq2", bufs=2)
                        nc.scalar.activation(sq2[:], xc[:, i, qcols],
                                             AF.Identity,
                                             accum_out=s2q[:, 0, qq:qq + 1])
                    stats2.append(s2q)

                # reduce over the 16 channels of each group: indicator matmul
                # ([32, 2, 4] per-quarter partials), then fold the quarters
                psg = ps_gn.tile([NG, 2, 4], F32, name="psg")
                for i in range(CC):
                    nc.tensor.matmul(psg[:], ind_sb[:, i, :], stats2[i][:],
                                     start=(i == 0), stop=(i == CC - 1))
                # ind is pre-scaled by 1/(GS*S) host-side, so psg already
                # holds per-quarter [mean, E[x^2]] contributions
                gstat = work.tile([NG, 2], F32, name="gstat")  # [mean, E[x^2]]
                nc.vector.tensor_reduce(out=gstat[:], in_=psg[:],
                                        axis=mybir.AxisListType.X, op=ALU.add)

                # rstd_g = (var+eps)^-0.5 via exp(-0.5*ln(var+eps)) -- Ln and
                # Exp share the already-loaded table set (Sqrt would force a
                # set switch) -- plus one Newton step for full fp32 accuracy
                nve = work.tile([NG, 1], F32, name="nve")  # mean^2 - E[x^2]
                nc.vector.scalar_tensor_tensor(
                    out=nve[:], in0=gstat[:, 0:1], scalar=gstat[:, 0:1],
                    in1=gstat[:, 1:2], op0=ALU.mult, op1=ALU.subtract)
                lnv = work.tile([NG, 1], F32, name="lnv")
                nc.scalar.activation(lnv[:], nve[:], AF.Ln, scale=-1.0,
                                     bias=eps_sb[:])
                r0 = work.tile([NG, 1], F32, name="r0")
                nc.scalar.activation(r0[:], lnv[:], AF.Exp, scale=-0.5)
                ve = work.tile([NG, 1], F32, name="ve")
                nc.scalar.activation(ve[:], nve[:], AF.Identity, scale=-1.0,
                                     bias=eps_sb[:])
                r0sq = work.tile([NG, 1], F32, name="r0sq")
                nc.vector.tensor_mul(r0sq[:], r0[:], r0[:])
                t2 = work.tile([NG, 1], F32, name="t2")
                nc.vector.tensor_mul(t2[:], ve[:], r0sq[:])
                t3 = work.tile([NG, 1], F32, name="t3")
                nc.vector.tensor_scalar(out=t3[:], in0=t2[:], scalar1=-0.5,
                                        scalar2=1.5, op0=ALU.mult, op1=ALU.add)
                gv = work.tile([NG, 2], F32, name="gv")  # [mean_g, rstd_g]
                nc.vector.tensor_copy(gv[:, 0:1], gstat[:, 0:1])
                nc.vector.tensor_mul(gv[:, 1:2], r0[:], t3[:])

                # broadcast group stats back to channels; sc = rstd*gamma
                # and bi' = mean*sc - beta read the broadcast PSUM directly
                # (h = x*sc - bi' on DVE chunks; ACT chunks negate the bias)
                sc_bi = []
                for i in range(CC):
                    psb = ps_gn.tile([128, 2], F32, name="psb")
                    nc.tensor.matmul(psb[:], indT_sb[:, i, :], gv[:],
                                     start=True, stop=True)
                    sc_c = work.tile([128, 1], F32, name="sc_c", tag="gn_sc", bufs=4)
                    nc.vector.tensor_mul(sc_c[:], psb[:, 1:2], ga_sb[:, i:i + 1])
                    bi_c = work.tile([128, 1], F32, name="bi_c", tag="gn_bi", bufs=4)
                    nc.vector.scalar_tensor_tensor(
                        out=bi_c[:], in0=psb[:, 0:1], scalar=sc_c[:],
                        in1=be_sb[:, i:i + 1], op0=ALU.mult, op1=ALU.subtract)
                    if i % 2 == 0:
                        bn_c = work.tile([128, 1], F32, name="bn_c",
                                         tag="gn_bn", bufs=2)
                        nc.vector.tensor_scalar_mul(bn_c[:], bi_c[:], -1.0)
                        sc_bi.append((sc_c, bn_c))
                    else:
                        sc_bi.append((sc_c, bi_c))

                # h = x*scale + bias, cast to fp8 -- split into halves and
                # alternate ScalarE/VectorE; all first halves go before the
                # second halves so the projections (which consume 512-col
                # blocks in order) can start as early as possible
                for hh in range(2):
                    cols = slice(hh * SQ, (hh + 1) * SQ)
                    for i in range(CC):
                        sc_c, bi_c = sc_bi[i]
                        hslc = h8[:, i // 2, 4 * hh:4 * hh + 4, i % 2, :]
                        if i % 2 == 0:
                            nc.scalar.activation(hslc,
                                                 xc[:, i, cols], AF.Identity,
                                                 bias=bi_c[:], scale=sc_c[:])
                        else:
                            nc.vector.tensor_scalar(
                                out=hslc, in0=xc[:, i, cols],
                                scalar1=sc_c[:], scalar2=bi_c[:],
                                op0=ALU.mult, op1=ALU.subtract)

            # =========== Phase 2: k'/v' projections ===========
            with tc.tile_pool(name="ps_proj", bufs=3, space="PSUM") as ps_p:
                # k' = M h, stored in the same interleaved fp8 layout as h8
                # (out-chunk oc -> (u=oc//2, j=oc%2)) so the scores matmul
                # can run DoubleRow. PSUM evacuations alternate ScalarE/DVE
                # so neither engine becomes the phase bottleneck.
                for oc in range(CC):
                    for sb in range(SB):
                        pt = ps_p.tile([128, 512], F32, name="pt", tag="pp")
                        for u in range(2):
                            nc.tensor.matmul(
                                pt[:],
                                w8_sb["w8m"][:, u, :, oc * 128:(oc + 1) * 128],
                                h8[:, u, sb, :, :],
                                start=(u == 0), stop=(u == 1), perf_mode=DR)
                        dst = kp8[:, oc // 2, sb, oc % 2, :]
                        if (oc * SB + sb) % 2 == 0:
                            nc.scalar.copy(dst, pt[:])
                        else:
                            nc.vector.tensor_copy(dst, pt[:])
                # vT'[s, c] = h[:, s]^T Wov^T  (spatial on partitions)
                for st in range(ST):
                    pt = ps_p.tile([128, 512], F32, name="pt", tag="pp")
                    ccol = slice((st % 4) * 128, (st % 4) * 128 + 128)
                    for u in range(2):
                        nc.tensor.matmul(pt[:], h8[:, u, st // 4, :, ccol],
                                         w8_sb["w8ov"][:, u, :, :],
                                         start=(u == 0), stop=(u == 1),
                                         perf_mode=DR)
                    if st % 2 == 0:
                        nc.scalar.copy(vT8[:, st, :], pt[:])
                    else:
                        nc.vector.tensor_copy(vT8[:, st, :], pt[:])

            # =========== Phase 3: attention ===========
            with tc.tile_pool(name="ps_po", bufs=4, space="PSUM") as ps_po, \
                 tc.tile_pool(name="ps_z", bufs=1, space="PSUM") as ps_z, \
                 tc.tile_pool(name="ps_s", bufs=3, space="PSUM") as ps_s:

                NP = ST // 2   # key-tile pairs (fp8 DoubleRow packs 2)

                def emit_scores_pair(qb, t):
                    e8p = work.tile([128, 2, 512], F8, name="e8p",
                                    tag="e8p", bufs=3)
                    for j in range(2):
                        st = 2 * t + j
                        pscore = ps_s.tile([128, 512], F32, name="pscore",
                                           tag="msum")
                        sc128 = slice((st % 4) * 128, (st % 4) * 128 + 128)
                        for u in range(2):
                            nc.tensor.matmul(
                                pscore[:], kp8[:, u, st // 4, :, sc128],
                                h8[:, u, qb, :, :],
                                start=(u == 0), stop=(u == 1), perf_mode=DR)
                        # e' = exp(score/sqrt(C)) * 2^-4 so fp8e4m3 never
                        # overflows; the shift cancels against Z in the
                        # final normalization
                        nc.scalar.activation(e8p[:, j, :], pscore[:], AF.Exp,
                                             scale=SCALE, bias=e8b_sb[:])
                    return e8p

                def emit_av(po, pz, t, e8p):
                    for cc2 in range(CC):
                        nc.tensor.matmul(
                            po[cc2][:],
                            vT8[:, 2 * t:2 * t + 2, cc2 * 128:(cc2 + 1) * 128],
                            e8p[:],
                            start=(t == 0), stop=(t == NP - 1), perf_mode=DR)
                    nc.tensor.matmul(pz[:], ones8[:], e8p[:],
                                     start=(t == 0), stop=(t == NP - 1),
                                     perf_mode=DR)

                for qb in range(QB):
                    po = [ps_po.tile([128, 512], F32, name="po", tag="po")
                          for _ in range(CC)]
                    pz = ps_z.tile([128, 512], F32, name="pz", tag="pz")
                    # software-pipelined: scores/exp for pair t+1 are
                    # issued before the AV matmuls of pair t, so the PE
                    # never waits on the ScalarE exp.
                    e_prev = emit_scores_pair(qb, 0)
                    for t in range(1, NP):
                        e_cur = emit_scores_pair(qb, t)
                        emit_av(po, pz, t - 1, e_prev)
                        e_prev = e_cur
                    emit_av(po, pz, NP - 1, e_prev)
                    # normalize + bias + residual + writeout, all on DVE off
                    # the PE critical path (the 2^-4 shift cancels po/Z)
                    qcols = slice(qb * 512, (qb + 1) * 512)
                    rzb = work.tile([128, 512], F32, name="rzb", tag="rzb",
                                    bufs=2)
                    nc.vector.reciprocal(rzb[:], pz[:])
                    for oc in range(CC):
                        t32 = work.tile([128, 512], F32, name="t32",
                                        tag="t32", bufs=2)
                        nc.vector.tensor_mul(t32[:], po[oc][:], rzb[:])
                        o16 = work.tile([128, 512], F16, name="o16",
                                        tag="o16", bufs=2)
                        nc.vector.scalar_tensor_tensor(
                            out=o16[:], in0=t32[:], scalar=bo_sb[:, oc:oc + 1],
                            in1=xc[:, oc, qcols], op0=ALU.add, op1=ALU.add)
                        nc.sync.dma_start(out=out_d[oc, :, qcols], in_=o16[:])

    _split_excess_waits(nc)
    return nc


_cache = {}


def _get_program():
    if "nc" not in _cache:
        _cache["nc"] = _build()
    return _cache["nc"]


def kernel(x, gamma, beta, wq, bq, wk, bk, wv, bv, wo, bo, trace=False):
    x = np.asarray(x, dtype=np.float32)
    gamma = np.asarray(gamma, dtype=np.float32)
    beta = np.asarray(beta, dtype=np.float32)
    wq, wk, wv, wo = (np.asarray(a, dtype=np.float32) for a in (wq, wk, wv, wo))
    bq, bk, bv, bo = (np.asarray(a, dtype=np.float32) for a in (bq, bk, bv, bo))
    assert not (np.any(bq) or np.any(bk)), \
        "nonzero bq/bk not supported by the fused-scores fast path"

    nc = _get_program()

    f8np = mybir.dt.np(F8)

    def pack8(w):
        wt = np.ascontiguousarray(w.T.astype(np.float32))
        return np.ascontiguousarray(
            wt.reshape(2, 2, 128, C).transpose(2, 0, 1, 3)).astype(f8np)

    # fold the two q/k projections into M and the v/out projections into
    # Wov; bv rides along as a constant output offset (sum_s w[s,q] = 1)
    M = wq.T @ wk
    Wov = wo @ wv
    bo_eff = wo @ bv + bo

    shared = {
        "w8m": pack8(M), "w8ov": pack8(Wov),
        "boc": np.ascontiguousarray(bo_eff.reshape(CC, 128).T),
        "gammac": np.ascontiguousarray(gamma.reshape(CC, 128).T),
        "betac": np.ascontiguousarray(beta.reshape(CC, 128).T),
    }
    ind = np.zeros((128, CC, NG), np.float32)
    indT = np.zeros((NG, CC, 128), np.float32)
    for i in range(CC):
        for p in range(128):
            g = (i * 128 + p) // GS
            ind[p, i, g] = 1.0 / (GS * S)
            indT[g, i, p] = 1.0
    shared["ind"] = ind
    shared["indT"] = indT

    in_maps = []
    for core in range(8):
        b, half = core // 2, core % 2
        xs = x[b].reshape(C, S)
        if half:
            xin = np.concatenate([xs[:, SQ:], xs[:, :SQ]], axis=1)
        else:
            xin = np.ascontiguousarray(xs)
        in_maps.append({"x16": xin.astype(np.float16), **shared})

    res = run_bass_kernel_spmd(nc, in_maps, core_ids=list(range(8)),
                               trace=trace)
    _cache["last_exec_time_ns"] = res.exec_time_ns

    y = np.empty((B, C, S), np.float32)
    for core in range(8):
        b, half = core // 2, core % 2
        y[b, :, half * SQ:(half + 1) * SQ] = \
            res.results[core]["out"].reshape(C, SQ).astype(np.float32)
    return y.reshape(B, C, H, W)
